# revision 15
# baseline (speedup 1.0000x reference)
"""DCellNet (hierarchical MLP with per-term BatchNorm) on 8 Trainium2 cores.

Sharding: expert-style over the 384 leaf terms -> 48 leaves (and therefore
2 mid terms) per core, fully local through the mid level (BatchNorm is exact
because every core holds the full batch for its own terms).  The root level
is sharded over its 900 output features (113 per core, padded to 904) after
an AllGather of the 16 mid activations; aux_r needs one tiny AllReduce.

All tensors on device live feature-on-partition (transposed vs. the
reference layout), so BatchNorm reductions run along the free axis and the
per-feature affine uses native per-partition scalars.  Matmuls run in
float32r (TF32-like, full PE rate at N>=256, ~2e-4 relative error).
"""

import ml_dtypes
import numpy as np

import concourse.bacc as bacc
import concourse.mybir as mybir
import concourse.tile as tile
from concourse.tile_rust import add_dep_helper
from concourse import bass_utils

F32 = mybir.dt.float32
F32R = mybir.dt.float32r
BF16 = mybir.dt.bfloat16
AF = mybir.ActivationFunctionType
OP = mybir.AluOpType

N_CORES = 8
L, M, C, G = 384, 16, 24, 10
NF, B = 3000, 2048
DL, DM, DR = 15, 72, 900
EPS = 1e-5

NFP = 3072            # NF padded to 24 k-tiles of 128
KT1 = NFP // 128      # 24
Lc = L // N_CORES     # 48 leaves per core
LGc = Lc * G          # 480 d-rows per core
LDc = Lc * DL         # 720 z-rows per core
Mc = M // N_CORES     # 2 mids per core
DRp = 904             # DR padded to 8*113
DRc = DRp // N_CORES  # 113 root features per core

CW1 = 256             # phase-1 batch chunk (fp32r needs N>=256 for full rate)
NB1 = B // CW1        # 8
CW2 = 512             # downstream batch chunk
NB2 = B // CW2        # 4

# Row-blocking of the 720 leaf-activation rows: per 120-row d-tile (12
# leaves) the leaf Linear produces 180 rows, split 128+52 to fit the PSUM
# partition limit.  (start, size) in the 720-row stack:
RB = []
for _mt in range(4):
    RB.append((180 * _mt, 128))
    RB.append((180 * _mt + 128, 52))

_CACHE = {}


def _build_program():
    nc = bacc.Bacc("TRN2", target_bir_lowering=False, debug=False,
                   num_devices=N_CORES)

    # ---------------- DRAM I/O ----------------
    di = {}

    def inp(name, shape, dt=F32):
        di[name] = nc.dram_tensor(name, list(shape), dt, kind="ExternalInput")
        return di[name]

    def outp(name, shape, dt=F32):
        di[name] = nc.dram_tensor(name, list(shape), dt, kind="ExternalOutput")
        return di[name]

    inp("xt", [NB1, 128, KT1, CW1], F32R)
    inp("wd", [NFP, LGc], F32R)
    inp("wla", [4, 120, 128], F32R)
    inp("wlb", [4, 120, 52], F32R)
    inp("blp", [LDc, 1])
    inp("gl", [LDc, 1])
    inp("bel", [LDc, 1])
    inp("wa1a", [4, 128, Lc], F32R)
    inp("wa1b", [4, 52, Lc], F32R)
    inp("ba1l", [Lc, 1])
    inp("wa2l", [Lc, 1])
    inp("ba2l", [Lc, 1])
    inp("wma", [Mc, 2, 128, DM], F32R)
    inp("wmb", [Mc, 2, 52, DM], F32R)
    inp("bm", [Mc * DM, 1])
    inp("gm", [Mc * DM, 1])
    inp("bem", [Mc * DM, 1])
    inp("wa1m", [Mc, DM, 1], F32R)
    inp("ba1m", [Mc, 1])
    inp("wa2m", [Mc, 1])
    inp("ba2m", [Mc, 1])
    inp("wr", [9, 128, DRc], F32R)
    inp("br", [DRc, 1])
    inp("gr", [DRc, 1])
    inp("ber", [DRc, 1])
    inp("wa1r", [DRc, 1], F32R)
    inp("ba1r", [1, 1])
    inp("wa2r", [1, 1])
    inp("ba2r", [1, 1])

    outp("hl", [LDc, B])
    outp("auxl", [Lc, B])
    outp("hm", [Mc * DM, B])
    outp("auxm", [Mc, B])
    outp("hr", [DRc, B])
    outp("auxr", [1, B])
    outp("wrm", [128, 128])

    rg = [list(range(N_CORES))]

    with tile.TileContext(nc) as tc:
        # ---------------- persistent small weights + BN scratch ----------
        p_w = tc.alloc_tile_pool(name="w", bufs=1)

        def load(name, src_ap, shape, dt=F32):
            t = p_w.tile(list(shape), dt, tag=name, name=name)
            nc.sync.dma_start(t, src_ap)
            return t

        wla_sb = [load(f"wla{i}", di["wla"].ap()[i], [120, 128], F32R) for i in range(4)]
        wlb_sb = [load(f"wlb{i}", di["wlb"].ap()[i], [120, 52], F32R) for i in range(4)]
        blp_s = []
        for i, (r0, sz) in enumerate(RB):
            blp_s.append(load(f"blp{i}", di["blp"].ap()[r0:r0 + sz, :], [sz, 1]))

        # BN pieces. Stats (bn_stats per 512-chunk) are emitted near the
        # producer of each chunk; the scalar chain + apply run per block.
        def bn_alloc_stats(P, nm):
            return p_w.tile([P, 24], F32, tag=f"bst_{nm}", bufs=1,
                            name=f"bst_{nm}")

        def bn_chunk_stats(st, t_ap, c):
            t32 = t_ap.bitcast(F32)
            nc.vector.bn_stats(st[:, 6 * c:6 * c + 6],
                               t32[:, 512 * c:512 * (c + 1)])

        def bn_var_eps(st, P, nm):
            mv = p_w.tile([P, 2], F32, tag="bn_mv", bufs=8, name=f"mv_{nm}")
            nc.vector.bn_aggr(mv, st)
            ve = p_w.tile([P, 1], F32, tag="bn_ve", bufs=8, name=f"ve_{nm}")
            nc.vector.tensor_scalar_add(ve, mv[:, 1:2], EPS)
            return mv, ve

        def bn_sqrt(ve, P, nm):
            sd = p_w.tile([P, 1], F32, tag="bn_sd", bufs=8, name=f"sd_{nm}")
            nc.scalar.activation(sd, ve, AF.Sqrt, bias=0.0)
            return sd

        def bn_finish(mv, sd, g_ap, be_ap, P, nm):
            ri = p_w.tile([P, 1], F32, tag="bn_ri", bufs=8, name=f"ri_{nm}")
            nc.vector.reciprocal(ri, sd)
            sc = p_w.tile([P, 1], F32, tag="bn_sc", bufs=8, name=f"sc_{nm}")
            nc.vector.tensor_tensor(sc, ri, g_ap, op=OP.mult)
            tp = p_w.tile([P, 1], F32, tag="bn_tp", bufs=8, name=f"tp_{nm}")
            nc.vector.tensor_tensor(tp, mv[:, 0:1], sc, op=OP.mult)
            sh = p_w.tile([P, 1], F32, tag="bn_sh", bufs=8, name=f"sh_{nm}")
            nc.vector.tensor_tensor(sh, be_ap, tp, op=OP.subtract)
            return sc, sh

        # ---------------- phase 1 + 2: x @ Wd, leaf Linear, tanh ----------
        p_th = tc.alloc_tile_pool(name="th", bufs=1, side="right")
        t_sb = [p_th.tile([RB[i][1], B], F32R, tag=f"t{i}", name=f"t{i}")
                for i in range(8)]

        p_xd = tc.alloc_tile_pool(name="xd", bufs=1)
        wd_sb = [p_xd.tile([128, LGc], F32R, tag=f"wdk{k}", name=f"wdk{k}")
                 for k in range(KT1)]
        nc.sync.dma_start(wd_sb[0], di["wd"].ap()[0:128, :])

        stl = [bn_alloc_stats(RB[i][1], f"l{i}") for i in range(8)]
        pp_warm = tc.alloc_tile_pool(name="ppw", bufs=1, space="PSUM")
        warm_ps = pp_warm.tile([128, CW2], F32, tag="warm", bufs=1,
                               name="warm_ps")
        pp1 = tc.alloc_tile_pool(name="pp1", bufs=1, space="PSUM")
        for b in range(NB1):
            c0 = CW1 * b
            xt_t = p_xd.tile([128, KT1, CW1], F32R, tag="xt", bufs=2, name="xt")
            nc.sync.dma_start(xt_t, di["xt"].ap()[b])
            if b == 0:
                for k in range(1, KT1):
                    nc.sync.dma_start(wd_sb[k],
                                      di["wd"].ap()[128 * k:128 * (k + 1), :])
            d_cur = []
            for m in range(4):
                ps1 = pp1.tile([120, CW1], F32, tag="p1", bufs=3, name="ps1")
                for k in range(KT1):
                    nc.tensor.matmul(ps1, wd_sb[k][:, 120 * m:120 * (m + 1)],
                                     xt_t[:, k, :],
                                     start=(k == 0), stop=(k == KT1 - 1))
                d_t = p_xd.tile([120, CW1], F32R, tag=f"d{m}", bufs=2, name=f"d{m}")
                nc.vector.tensor_copy(d_t, ps1)
                d_cur.append(d_t)
            for mt in range(4):
                for part in range(2):
                    i = 2 * mt + part
                    sz = RB[i][1]
                    w = wla_sb[mt] if part == 0 else wlb_sb[mt]
                    psz = pp1.tile([sz, CW1], F32, tag=f"pz{part}", bufs=2,
                                   name=f"psz{part}")
                    last_z_mm = nc.tensor.matmul(psz, w, d_cur[mt],
                                                 start=True, stop=True)
                    nc.scalar.activation(t_sb[i][:, c0:c0 + CW1], psz, AF.Tanh,
                                         bias=blp_s[i])
            if b == 0:
                # preload the Sqrt ACT table while phase 1 runs
                sd_pre = p_w.tile([1, 1], F32, tag="sd_pre", name="sd_pre")
                nc.scalar.activation(sd_pre, blp_s[0][0:1, :], AF.Sqrt, bias=0.0)
            if b % 2 == 1:
                for i in range(8):
                    bn_chunk_stats(stl[i], t_sb[i], b // 2)

        pp1.release()
        p_xd.release()

        wa1_sb = []
        for i in range(8):
            mt, part = i // 2, i % 2
            src = di["wa1a"].ap()[mt] if part == 0 else di["wa1b"].ap()[mt]
            sz = RB[i][1]
            wa1_sb.append(load(f"wa1_{i}", src, [sz, Lc], F32R))
        wm_sb = {}
        for m in range(Mc):
            for j in range(4):
                src = di["wma"].ap()[m, j // 2] if j % 2 == 0 else di["wmb"].ap()[m, j // 2]
                sz = 128 if j % 2 == 0 else 52
                wm_sb[(m, j)] = load(f"wm_{m}_{j}", src, [sz, DM], F32R)

        gl_s, bel_s = [], []
        for i, (r0, sz) in enumerate(RB):
            gl_s.append(load(f"gl{i}", di["gl"].ap()[r0:r0 + sz, :], [sz, 1]))
            bel_s.append(load(f"bel{i}", di["bel"].ap()[r0:r0 + sz, :], [sz, 1]))
        ba1l_sb = load("ba1l", di["ba1l"].ap(), [Lc, 1])
        wa2l_sb = load("wa2l", di["wa2l"].ap(), [Lc, 1])
        ba2l_sb = load("ba2l", di["ba2l"].ap(), [Lc, 1])
        bm_s = [load(f"bm{m}", di["bm"].ap()[DM * m:DM * (m + 1), :], [DM, 1]) for m in range(Mc)]
        gm_s = [load(f"gm{m}", di["gm"].ap()[DM * m:DM * (m + 1), :], [DM, 1]) for m in range(Mc)]
        bem_s = [load(f"bem{m}", di["bem"].ap()[DM * m:DM * (m + 1), :], [DM, 1]) for m in range(Mc)]
        wa1m_s = [load(f"wa1m{m}", di["wa1m"].ap()[m], [DM, 1], F32R) for m in range(Mc)]
        ba1m_s = [load(f"ba1m{m}", di["ba1m"].ap()[m:m + 1, :], [1, 1]) for m in range(Mc)]
        wa2m_s = [load(f"wa2m{m}", di["wa2m"].ap()[m:m + 1, :], [1, 1]) for m in range(Mc)]
        ba2m_s = [load(f"ba2m{m}", di["ba2m"].ap()[m:m + 1, :], [1, 1]) for m in range(Mc)]
        br_sb = load("br", di["br"].ap(), [DRc, 1])
        gr_sb = load("gr", di["gr"].ap(), [DRc, 1])
        ber_sb = load("ber", di["ber"].ap(), [DRc, 1])
        wa1r_sb = load("wa1r", di["wa1r"].ap(), [DRc, 1], F32R)
        ba1r_sb = load("ba1r", di["ba1r"].ap(), [1, 1])
        wa2r_sb = load("wa2r", di["wa2r"].ap(), [1, 1])
        ba2r_sb = load("ba2r", di["ba2r"].ap(), [1, 1])


        # ---------------- leaf BN -> h_l, in place (f32r view) ------------
        h_sb = []
        mv_l, ve_l, sd_l, schl = [], [], [], []
        for i in range(8):
            mv, ve = bn_var_eps(stl[i], RB[i][1], f"l{i}")
            mv_l.append(mv)
            ve_l.append(ve)
        for i in range(8):
            sd_l.append(bn_sqrt(ve_l[i], RB[i][1], f"l{i}"))
        for i in range(8):
            schl.append(bn_finish(mv_l[i], sd_l[i], gl_s[i], bel_s[i],
                                  RB[i][1], f"l{i}"))
        for c in range(4):
            for i in range(8):
                sc, sh = schl[i]
                cc0 = 512 * c
                nc.vector.tensor_scalar(t_sb[i][:, cc0:cc0 + 512],
                                        t_sb[i][:, cc0:cc0 + 512], sc, sh,
                                        op0=OP.mult, op1=OP.add)
        for i in range(8):
            h_sb.append(t_sb[i])

        # ---------------- mid level ----------------
        p_m = tc.alloc_tile_pool(name="mid", bufs=1)
        p_dram = tc.alloc_tile_pool(name="dram", bufs=1, space="DRAM")
        ccin = p_dram.tile([Mc * DM, B], F32, name="ccin")
        ccout = p_dram.tile([M * DM, B], F32, addr_space="Shared", name="ccout")
        pp2 = tc.alloc_tile_pool(name="pp2", bufs=1, space="PSUM")

        hmr = []
        last_ccin = None
        for m in range(Mc):
            tm = p_m.tile([DM, B], F32R, tag="tm", bufs=2, name=f"tm{m}")
            stm = bn_alloc_stats(DM, f"m{m}")
            for b in range(NB2):
                c0 = CW2 * b
                psm = pp2.tile([DM, CW2], F32, tag="pm", bufs=2, name="psm")
                for j in range(4):
                    nc.tensor.matmul(psm, wm_sb[(m, j)],
                                     h_sb[4 * m + j][:, c0:c0 + CW2],
                                     start=(j == 0), stop=(j == 3))
                nc.scalar.activation(tm[:, c0:c0 + CW2], psm, AF.Tanh, bias=bm_s[m])
                bn_chunk_stats(stm, tm, b)
            mv, ve = bn_var_eps(stm, DM, f"m{m}")
            sd = bn_sqrt(ve, DM, f"m{m}")
            sc, sh = bn_finish(mv, sd, gm_s[m], bem_s[m], DM, f"m{m}")
            for b in range(NB2):
                c0 = CW2 * b
                nc.vector.tensor_scalar(tm[:, c0:c0 + CW2], tm[:, c0:c0 + CW2],
                                        sc, sh, op0=OP.mult, op1=OP.add)
                last_ccin = nc.sync.dma_start(
                    ccin[DM * m:DM * (m + 1), c0:c0 + CW2].bitcast(F32R),
                    tm[:, c0:c0 + CW2])
            hmr.append(tm)

        nc.gpsimd.collective_compute(
            "AllGather", OP.bypass, replica_groups=rg,
            ins=[ccin.opt()], outs=[ccout.opt()],
        )

        # root weights load early (overlaps the AllGather)
        p_wr = tc.alloc_tile_pool(name="wr", bufs=1)
        wr_sb = [p_wr.tile([128, DRc], F32R, tag=f"wrk{k}", name=f"wrk{k}")
                 for k in range(9)]
        for k in range(9):
            nc.sync.dma_start(wr_sb[k], di["wr"].ap()[k])

        # ---------------- deferred outputs + aux (overlap the AllGather) --
        for i, (r0, sz) in enumerate(RB):
            dd = nc.scalar.dma_start(di["hl"].ap()[r0:r0 + sz, :].bitcast(F32R),
                                     h_sb[i])
            add_dep_helper(dd.ins, last_ccin.ins, sync=True,
                           reason="outputs queue after collective feeders")
        for m in range(Mc):
            dd = nc.scalar.dma_start(
                di["hm"].ap()[DM * m:DM * (m + 1), :].bitcast(F32R), hmr[m])
            add_dep_helper(dd.ins, last_ccin.ins, sync=True,
                           reason="outputs queue after collective feeders")

        aux1l = p_m.tile([Lc, B], F32, tag="aux1l", name="aux1l")
        for b in range(NB2):
            c0 = CW2 * b
            psa = pp2.tile([Lc, CW2], F32, tag="pa", bufs=2, name="psa")
            for i in range(8):
                amm = nc.tensor.matmul(psa, wa1_sb[i], h_sb[i][:, c0:c0 + CW2],
                                       start=(i == 0), stop=(i == 7))
                if b == 0 and i == 0:
                    add_dep_helper(amm.ins, last_ccin.ins, sync=True,
                                   reason="aux fills the AllGather window")
            nc.scalar.activation(aux1l[:, c0:c0 + CW2], psa, AF.Tanh, bias=ba1l_sb)
        auxl32 = p_m.tile([Lc, B], F32, tag="auxl32", name="auxl32")
        nc.vector.tensor_scalar(auxl32, aux1l, wa2l_sb, ba2l_sb,
                                op0=OP.mult, op1=OP.add)
        nc.scalar.dma_start(di["auxl"].ap(), auxl32)

        for m in range(Mc):
            aux1m = p_m.tile([1, B], F32, tag="aux1m", bufs=1, name="aux1m")
            for b in range(NB2):
                c0 = CW2 * b
                psam = pp2.tile([1, CW2], F32, tag="pam", bufs=2, name="psam")
                nc.tensor.matmul(psam, wa1m_s[m], hmr[m][:, c0:c0 + CW2],
                                 start=True, stop=True)
                nc.scalar.activation(aux1m[:, c0:c0 + CW2], psam, AF.Tanh,
                                     bias=ba1m_s[m])
            auxm32 = p_m.tile([1, B], F32, tag="auxm32", bufs=1, name="auxm32")
            nc.vector.tensor_scalar(auxm32, aux1m, wa2m_s[m], ba2m_s[m],
                                    op0=OP.mult, op1=OP.add)
            nc.scalar.dma_start(di["auxm"].ap()[m:m + 1, :], auxm32)

        pp2.release()
        p_th.release()

        # ---------------- root level (feature-sharded) ----------------
        p_rt = tc.alloc_tile_pool(name="rt", bufs=1)
        pp3 = tc.alloc_tile_pool(name="pp3", bufs=1, space="PSUM")

        tr = p_rt.tile([DRc, B], F32R, tag="tr", name="tr")
        str_ = bn_alloc_stats(DRc, "r")
        psrs = [pp3.tile([DRc, CW2], F32, tag="pr", bufs=4, name=f"psr{b}")
                for b in range(NB2)]
        for ki in range(9):
            hma = p_rt.tile([128, B], F32R, tag=f"hma{ki}", bufs=1,
                            name=f"hma{ki}")
            eng = nc.sync if ki % 2 == 0 else nc.scalar
            eng.dma_start(hma, ccout[128 * ki:128 * (ki + 1), :].bitcast(F32R))
            for b in range(NB2):
                c0 = CW2 * b
                nc.tensor.matmul(psrs[b], wr_sb[ki], hma[:, c0:c0 + CW2],
                                 start=(ki == 0), stop=(ki == 8))
        for b in range(NB2):
            c0 = CW2 * b
            nc.scalar.activation(tr[:, c0:c0 + CW2], psrs[b], AF.Tanh, bias=br_sb)
            bn_chunk_stats(str_, tr, b)
        mv, ve = bn_var_eps(str_, DRc, "r")
        sd = bn_sqrt(ve, DRc, "r")
        sc, sh = bn_finish(mv, sd, gr_sb, ber_sb, DRc, "r")
        nc.vector.tensor_scalar(tr, tr, sc, sh, op0=OP.mult, op1=OP.add)
        hrr = tr

        # aux_r partial + AllReduce trigger first; h_r output DMA overlaps it
        arp = p_rt.tile([1, B], F32, tag="arp", name="arp")
        for b in range(NB2):
            c0 = CW2 * b
            psar = pp3.tile([1, CW2], F32, tag="par", bufs=2, name="psar")
            nc.tensor.matmul(psar, wa1r_sb, hrr[:, c0:c0 + CW2],
                             start=True, stop=True)
            nc.vector.tensor_copy(arp[:, c0:c0 + CW2], psar)
        arin = p_dram.tile([1, B], F32, name="arin")
        arout = p_dram.tile([1, B], F32, addr_space="Shared", name="arout")
        nc.sync.dma_start(arin, arp)
        nc.gpsimd.collective_compute(
            "AllReduce", OP.add, replica_groups=rg,
            ins=[arin.opt()], outs=[arout.opt()],
        )
        nc.scalar.dma_start(di["hr"].ap().bitcast(F32R), hrr)
        arsb = p_rt.tile([1, B], F32, tag="arsb", name="arsb")
        nc.sync.dma_start(arsb, arout)
        aux1r = p_rt.tile([1, B], F32, tag="aux1r", name="aux1r")
        nc.scalar.activation(aux1r, arsb, AF.Tanh, bias=ba1r_sb)
        auxr32 = p_rt.tile([1, B], F32, tag="auxr32", name="auxr32")
        nc.vector.tensor_scalar(auxr32, aux1r, wa2r_sb, ba2r_sb,
                                op0=OP.mult, op1=OP.add)
        nc.scalar.dma_start(di["auxr"].ap(), auxr32)

        # Lowest-priority keep-warm matmuls: the scheduler slots these into
        # PE-idle windows (BN chain, AllGather, AllReduce waits) so the HAM
        # clock gate stays open; they do no useful work.
        warm_in = p_w.tile([120, LGc], F32R, tag="warm_in", name="warm_in")
        nc.sync.dma_start(warm_in, di["wd"].ap()[0:120, :])
        for wi in range(110):
            wmm = nc.tensor.matmul(warm_ps[:, 0:LGc], wla_sb[0], warm_in,
                                   start=True, stop=True)
            if wi == 0:
                add_dep_helper(wmm.ins, last_z_mm.ins, sync=True,
                               reason="warm fillers start after phase-1")
            if wi == 50:
                add_dep_helper(wmm.ins, last_ccin.ins, sync=True,
                               reason="second warm tier covers the AllGather")
        warm_sb = p_w.tile([128, 128], F32, tag="warm_sb", name="warm_sb")
        nc.vector.tensor_copy(warm_sb, warm_ps[:, 0:128])
        nc.scalar.dma_start(di["wrm"].ap(), warm_sb)

        pp3.release()
        pp_warm.release()
        p_rt.release()
        p_wr.release()
        p_dram.release()
        p_m.release()
        p_w.release()

    nc.compile()
    return nc


def _prep_in_maps(inputs):
    f32 = np.float32
    x = np.asarray(inputs["x"], f32)
    Wd = np.asarray(inputs["Wd"], f32)
    bd = np.asarray(inputs["bd"], f32)
    Wl = np.asarray(inputs["Wl"], f32)
    bl = np.asarray(inputs["bl"], f32)
    gl = np.asarray(inputs["gl"], f32)
    betal = np.asarray(inputs["betal"], f32)
    Wa1l = np.asarray(inputs["Wa1l"], f32)
    ba1l = np.asarray(inputs["ba1l"], f32)
    Wa2l = np.asarray(inputs["Wa2l"], f32)
    ba2l = np.asarray(inputs["ba2l"], f32)
    Wm = np.asarray(inputs["Wm"], f32)
    bm = np.asarray(inputs["bm"], f32)
    gm = np.asarray(inputs["gm"], f32)
    betam = np.asarray(inputs["betam"], f32)
    Wa1m = np.asarray(inputs["Wa1m"], f32)
    ba1m = np.asarray(inputs["ba1m"], f32)
    Wa2m = np.asarray(inputs["Wa2m"], f32)
    ba2m = np.asarray(inputs["ba2m"], f32)
    Wr = np.asarray(inputs["Wr"], f32)
    br = np.asarray(inputs["br"], f32)
    gr = np.asarray(inputs["gr"], f32)
    betar = np.asarray(inputs["betar"], f32)
    Wa1r = np.asarray(inputs["Wa1r"], f32)
    ba1r = np.asarray(inputs["ba1r"], f32)
    Wa2r = np.asarray(inputs["Wa2r"], f32)
    ba2r = np.asarray(inputs["ba2r"], f32)

    xt_full = np.zeros((NFP, B), f32)
    xt_full[:NF] = x.T
    # [NFP, B] -> [nb, 128, kt, cw] so each chunk DMA is contiguous per partition
    xt = np.ascontiguousarray(
        xt_full.reshape(KT1, 128, NB1, CW1).transpose(2, 1, 0, 3))

    # root weights, feature-padded to 904 and column-sharded
    Wr_p = np.zeros((M * DM, DRp), f32)
    Wr_p[:, :DR] = Wr
    br_p = np.zeros(DRp, f32)
    br_p[:DR] = br
    gr_p = np.ones(DRp, f32)
    gr_p[:DR] = gr
    ber_p = np.zeros(DRp, f32)
    ber_p[:DR] = betar
    wa1r_p = np.zeros((DRp, 1), f32)
    wa1r_p[:DR] = Wa1r

    in_maps = []
    for core in range(N_CORES):
        l0 = Lc * core
        m0 = Mc * core
        sl = slice(l0, l0 + Lc)
        sm = slice(m0, m0 + Mc)

        wd = np.zeros((NFP, LGc), f32)
        wd[:NF] = Wd[sl].transpose(1, 0, 2).reshape(NF, LGc)

        # leaf Linear as 4 block-diagonal [120 -> 180] matmuls, split 128+52
        wla = np.zeros((4, 120, 128), f32)
        wlb = np.zeros((4, 120, 52), f32)
        for mt in range(4):
            blk = np.zeros((120, 180), f32)
            for j in range(12):
                blk[10 * j:10 * (j + 1), 15 * j:15 * (j + 1)] = Wl[l0 + 12 * mt + j]
            wla[mt] = blk[:, :128]
            wlb[mt] = blk[:, 128:]

        # combined bias: (bd @ Wl + bl) per leaf, flattened (leaf, dl)
        blp = (np.einsum("lg,lgd->ld", bd[sl], Wl[sl]) + bl[sl]).reshape(LDc, 1)

        # aux-1 weights as a [720, 48] block matrix, row-blocked per RB
        W48 = np.zeros((LDc, Lc), f32)
        for j in range(Lc):
            W48[DL * j:DL * (j + 1), j] = Wa1l[l0 + j, :, 0]
        wa1a = np.zeros((4, 128, Lc), f32)
        wa1b = np.zeros((4, 52, Lc), f32)
        for i, (r0, sz) in enumerate(RB):
            if i % 2 == 0:
                wa1a[i // 2] = W48[r0:r0 + sz]
            else:
                wa1b[i // 2] = W48[r0:r0 + sz]

        wma = np.zeros((Mc, 2, 128, DM), f32)
        wmb = np.zeros((Mc, 2, 52, DM), f32)
        for m in range(Mc):
            wmat = Wm[m0 + m]  # [360, 72]
            wma[m, 0] = wmat[0:128]
            wmb[m, 0] = wmat[128:180]
            wma[m, 1] = wmat[180:308]
            wmb[m, 1] = wmat[308:360]

        rs = slice(DRc * core, DRc * (core + 1))
        in_maps.append({
            "xt": xt,
            "wd": wd,
            "wla": wla,
            "wlb": wlb,
            "blp": blp,
            "gl": gl[sl].reshape(LDc, 1).astype(f32),
            "bel": betal[sl].reshape(LDc, 1).astype(f32),
            "wa1a": wa1a,
            "wa1b": wa1b,
            "ba1l": ba1l[sl].reshape(Lc, 1),
            "wa2l": Wa2l[sl, 0, :].reshape(Lc, 1),
            "ba2l": ba2l[sl].reshape(Lc, 1),
            "wma": wma,
            "wmb": wmb,
            "bm": bm[sm].reshape(Mc * DM, 1),
            "gm": gm[sm].reshape(Mc * DM, 1),
            "bem": betam[sm].reshape(Mc * DM, 1),
            "wa1m": Wa1m[sm],
            "ba1m": ba1m[sm].reshape(Mc, 1),
            "wa2m": Wa2m[sm, 0, :].reshape(Mc, 1),
            "ba2m": ba2m[sm].reshape(Mc, 1),
            "wr": np.ascontiguousarray(Wr_p[:, rs]).reshape(9, 128, DRc),
            "br": br_p[rs].reshape(DRc, 1),
            "gr": gr_p[rs].reshape(DRc, 1),
            "ber": ber_p[rs].reshape(DRc, 1),
            "wa1r": np.ascontiguousarray(wa1r_p[rs]),
            "ba1r": ba1r.reshape(1, 1),
            "wa2r": Wa2r.reshape(1, 1)[:1, :1],
            "ba2r": ba2r.reshape(1, 1),
        })
    return in_maps


def _get_program():
    if "nc" not in _CACHE:
        _CACHE["nc"] = _build_program()
    return _CACHE["nc"]


def kernel(**inputs):
    nc = _get_program()
    in_maps = _prep_in_maps(inputs)
    res = bass_utils.run_bass_kernel_spmd(nc, in_maps,
                                          core_ids=list(range(N_CORES)))
    rs = res.results
    f32 = np.float32

    h_l = np.concatenate([r["hl"].reshape(Lc, DL, B) for r in rs], 0)
    h_l = np.ascontiguousarray(h_l.transpose(0, 2, 1), f32)
    aux_l = np.ascontiguousarray(
        np.concatenate([r["auxl"] for r in rs], 0), f32)[:, :, None]
    h_m = np.concatenate([r["hm"].reshape(Mc, DM, B) for r in rs], 0)
    h_m = np.ascontiguousarray(h_m.transpose(0, 2, 1), f32)
    aux_m = np.ascontiguousarray(
        np.concatenate([r["auxm"] for r in rs], 0), f32)[:, :, None]
    h_r = np.concatenate([r["hr"] for r in rs], 0)[:DR]
    h_r = np.ascontiguousarray(h_r.T, f32)
    aux_r = np.ascontiguousarray(rs[0]["auxr"].T, f32)
    return (aux_l, aux_m, aux_r, h_l, h_m, h_r)


# revision 16
# speedup vs baseline: 1.0112x; 1.0112x over previous
"""DCellNet (hierarchical MLP with per-term BatchNorm) on 8 Trainium2 cores.

Sharding: expert-style over the 384 leaf terms -> 48 leaves (and therefore
2 mid terms) per core, fully local through the mid level (BatchNorm is exact
because every core holds the full batch for its own terms).  The root level
is sharded over its 900 output features (113 per core, padded to 904) after
an AllGather of the 16 mid activations; aux_r needs one tiny AllReduce.

All tensors on device live feature-on-partition (transposed vs. the
reference layout), so BatchNorm reductions run along the free axis and the
per-feature affine uses native per-partition scalars.  Matmuls run in
float32r (TF32-like, full PE rate at N>=256, ~2e-4 relative error).
"""

import ml_dtypes
import numpy as np

import concourse.bacc as bacc
import concourse.mybir as mybir
import concourse.tile as tile
from concourse.tile_rust import add_dep_helper
from concourse import bass_utils

F32 = mybir.dt.float32
F32R = mybir.dt.float32r
BF16 = mybir.dt.bfloat16
AF = mybir.ActivationFunctionType
OP = mybir.AluOpType

N_CORES = 8
L, M, C, G = 384, 16, 24, 10
NF, B = 3000, 2048
DL, DM, DR = 15, 72, 900
EPS = 1e-5

NFP = 3072            # NF padded to 24 k-tiles of 128
KT1 = NFP // 128      # 24
Lc = L // N_CORES     # 48 leaves per core
LGc = Lc * G          # 480 d-rows per core
LDc = Lc * DL         # 720 z-rows per core
Mc = M // N_CORES     # 2 mids per core
DRp = 904             # DR padded to 8*113
DRc = DRp // N_CORES  # 113 root features per core

CW1 = 256             # phase-1 batch chunk (fp32r needs N>=256 for full rate)
NB1 = B // CW1        # 8
CW2 = 512             # downstream batch chunk
NB2 = B // CW2        # 4

# Row-blocking of the 720 leaf-activation rows: per 120-row d-tile (12
# leaves) the leaf Linear produces 180 rows, split 128+52 to fit the PSUM
# partition limit.  (start, size) in the 720-row stack:
RB = []
for _mt in range(4):
    RB.append((180 * _mt, 128))
    RB.append((180 * _mt + 128, 52))

_CACHE = {}


def _build_program():
    nc = bacc.Bacc("TRN2", target_bir_lowering=False, debug=False,
                   num_devices=N_CORES)

    # ---------------- DRAM I/O ----------------
    di = {}

    def inp(name, shape, dt=F32):
        di[name] = nc.dram_tensor(name, list(shape), dt, kind="ExternalInput")
        return di[name]

    def outp(name, shape, dt=F32):
        di[name] = nc.dram_tensor(name, list(shape), dt, kind="ExternalOutput")
        return di[name]

    inp("xt", [NB1, 128, KT1, CW1], F32R)
    inp("wd", [NFP, LGc], F32R)
    inp("wla", [4, 120, 128], F32R)
    inp("wlb", [4, 120, 52], F32R)
    inp("blp", [LDc, 1])
    inp("gl", [LDc, 1])
    inp("bel", [LDc, 1])
    inp("wa1a", [4, 128, Lc], F32R)
    inp("wa1b", [4, 52, Lc], F32R)
    inp("ba1l", [Lc, 1])
    inp("wa2l", [Lc, 1])
    inp("ba2l", [Lc, 1])
    inp("wma", [Mc, 2, 128, DM], F32R)
    inp("wmb", [Mc, 2, 52, DM], F32R)
    inp("bm", [Mc * DM, 1])
    inp("gm", [Mc * DM, 1])
    inp("bem", [Mc * DM, 1])
    inp("wa1m", [Mc, DM, 1], F32R)
    inp("ba1m", [Mc, 1])
    inp("wa2m", [Mc, 1])
    inp("ba2m", [Mc, 1])
    inp("wr", [9, 128, DRc], F32R)
    inp("br", [DRc, 1])
    inp("gr", [DRc, 1])
    inp("ber", [DRc, 1])
    inp("wa1r", [DRc, 1], F32R)
    inp("ba1r", [1, 1])
    inp("wa2r", [1, 1])
    inp("ba2r", [1, 1])

    outp("hl", [LDc, B])
    outp("auxl", [Lc, B])
    outp("hm", [Mc * DM, B])
    outp("auxm", [Mc, B])
    outp("hr", [DRc, B])
    outp("auxr", [1, B])
    outp("wrm", [128, 128])

    rg = [list(range(N_CORES))]

    with tile.TileContext(nc) as tc:
        # ---------------- persistent small weights + BN scratch ----------
        p_w = tc.alloc_tile_pool(name="w", bufs=1)

        def load(name, src_ap, shape, dt=F32):
            t = p_w.tile(list(shape), dt, tag=name, name=name)
            nc.sync.dma_start(t, src_ap)
            return t

        wla_sb = [load(f"wla{i}", di["wla"].ap()[i], [120, 128], F32R) for i in range(4)]
        wlb_sb = [load(f"wlb{i}", di["wlb"].ap()[i], [120, 52], F32R) for i in range(4)]
        blp_s = []
        for i, (r0, sz) in enumerate(RB):
            blp_s.append(load(f"blp{i}", di["blp"].ap()[r0:r0 + sz, :], [sz, 1]))

        # BN pieces. Stats (bn_stats per 512-chunk) are emitted near the
        # producer of each chunk; the scalar chain + apply run per block.
        def bn_alloc_stats(P, nm):
            return p_w.tile([P, 24], F32, tag=f"bst_{nm}", bufs=1,
                            name=f"bst_{nm}")

        def bn_chunk_stats(st, t_ap, c):
            t32 = t_ap.bitcast(F32)
            nc.vector.bn_stats(st[:, 6 * c:6 * c + 6],
                               t32[:, 512 * c:512 * (c + 1)])

        def bn_var_eps(st, P, nm):
            mv = p_w.tile([P, 2], F32, tag="bn_mv", bufs=8, name=f"mv_{nm}")
            nc.vector.bn_aggr(mv, st)
            ve = p_w.tile([P, 1], F32, tag="bn_ve", bufs=8, name=f"ve_{nm}")
            nc.vector.tensor_scalar_add(ve, mv[:, 1:2], EPS)
            return mv, ve

        def bn_sqrt(ve, P, nm):
            sd = p_w.tile([P, 1], F32, tag="bn_sd", bufs=8, name=f"sd_{nm}")
            nc.scalar.activation(sd, ve, AF.Sqrt, bias=0.0)
            return sd

        def bn_finish(mv, sd, g_ap, be_ap, P, nm):
            ri = p_w.tile([P, 1], F32, tag="bn_ri", bufs=8, name=f"ri_{nm}")
            nc.vector.reciprocal(ri, sd)
            sc = p_w.tile([P, 1], F32, tag="bn_sc", bufs=8, name=f"sc_{nm}")
            nc.vector.tensor_tensor(sc, ri, g_ap, op=OP.mult)
            tp = p_w.tile([P, 1], F32, tag="bn_tp", bufs=8, name=f"tp_{nm}")
            nc.vector.tensor_tensor(tp, mv[:, 0:1], sc, op=OP.mult)
            sh = p_w.tile([P, 1], F32, tag="bn_sh", bufs=8, name=f"sh_{nm}")
            nc.vector.tensor_tensor(sh, be_ap, tp, op=OP.subtract)
            return sc, sh

        # ---------------- phase 1 + 2: x @ Wd, leaf Linear, tanh ----------
        p_th = tc.alloc_tile_pool(name="th", bufs=1, side="right")
        t_sb = [p_th.tile([RB[i][1], B], F32R, tag=f"t{i}", name=f"t{i}")
                for i in range(8)]

        p_xd = tc.alloc_tile_pool(name="xd", bufs=1)
        wd_sb = [p_xd.tile([128, LGc], F32R, tag=f"wdk{k}", name=f"wdk{k}")
                 for k in range(KT1)]
        nc.sync.dma_start(wd_sb[0], di["wd"].ap()[0:128, :])

        stl = [bn_alloc_stats(RB[i][1], f"l{i}") for i in range(8)]
        pp_warm = tc.alloc_tile_pool(name="ppw", bufs=1, space="PSUM")
        warm_ps = pp_warm.tile([128, CW2], F32, tag="warm", bufs=1,
                               name="warm_ps")
        pp1 = tc.alloc_tile_pool(name="pp1", bufs=1, space="PSUM")
        for b in range(NB1):
            c0 = CW1 * b
            xt_t = p_xd.tile([128, KT1, CW1], F32R, tag="xt", bufs=2, name="xt")
            nc.sync.dma_start(xt_t, di["xt"].ap()[b])
            if b == 0:
                for k in range(1, KT1):
                    nc.sync.dma_start(wd_sb[k],
                                      di["wd"].ap()[128 * k:128 * (k + 1), :])
            d_cur = []
            for m in range(4):
                ps1 = pp1.tile([120, CW1], F32, tag="p1", bufs=3, name="ps1")
                for k in range(KT1):
                    nc.tensor.matmul(ps1, wd_sb[k][:, 120 * m:120 * (m + 1)],
                                     xt_t[:, k, :],
                                     start=(k == 0), stop=(k == KT1 - 1))
                d_t = p_xd.tile([120, CW1], F32R, tag=f"d{m}", bufs=2, name=f"d{m}")
                nc.vector.tensor_copy(d_t, ps1)
                d_cur.append(d_t)
            for mt in range(4):
                for part in range(2):
                    i = 2 * mt + part
                    sz = RB[i][1]
                    w = wla_sb[mt] if part == 0 else wlb_sb[mt]
                    psz = pp1.tile([sz, CW1], F32, tag=f"pz{part}", bufs=2,
                                   name=f"psz{part}")
                    last_z_mm = nc.tensor.matmul(psz, w, d_cur[mt],
                                                 start=True, stop=True)
                    nc.scalar.activation(t_sb[i][:, c0:c0 + CW1], psz, AF.Tanh,
                                         bias=blp_s[i])
            if b == 0:
                # preload the Sqrt ACT table while phase 1 runs
                sd_pre = p_w.tile([1, 1], F32, tag="sd_pre", name="sd_pre")
                nc.scalar.activation(sd_pre, blp_s[0][0:1, :], AF.Sqrt, bias=0.0)
            if b % 2 == 1:
                for i in range(8):
                    bn_chunk_stats(stl[i], t_sb[i], b // 2)

        pp1.release()
        p_xd.release()

        wa1_sb = []
        for i in range(8):
            mt, part = i // 2, i % 2
            src = di["wa1a"].ap()[mt] if part == 0 else di["wa1b"].ap()[mt]
            sz = RB[i][1]
            wa1_sb.append(load(f"wa1_{i}", src, [sz, Lc], F32R))
        wm_sb = {}
        for m in range(Mc):
            for j in range(4):
                src = di["wma"].ap()[m, j // 2] if j % 2 == 0 else di["wmb"].ap()[m, j // 2]
                sz = 128 if j % 2 == 0 else 52
                wm_sb[(m, j)] = load(f"wm_{m}_{j}", src, [sz, DM], F32R)

        gl_s, bel_s = [], []
        for i, (r0, sz) in enumerate(RB):
            gl_s.append(load(f"gl{i}", di["gl"].ap()[r0:r0 + sz, :], [sz, 1]))
            bel_s.append(load(f"bel{i}", di["bel"].ap()[r0:r0 + sz, :], [sz, 1]))
        ba1l_sb = load("ba1l", di["ba1l"].ap(), [Lc, 1])
        wa2l_sb = load("wa2l", di["wa2l"].ap(), [Lc, 1])
        ba2l_sb = load("ba2l", di["ba2l"].ap(), [Lc, 1])
        bm_s = [load(f"bm{m}", di["bm"].ap()[DM * m:DM * (m + 1), :], [DM, 1]) for m in range(Mc)]
        gm_s = [load(f"gm{m}", di["gm"].ap()[DM * m:DM * (m + 1), :], [DM, 1]) for m in range(Mc)]
        bem_s = [load(f"bem{m}", di["bem"].ap()[DM * m:DM * (m + 1), :], [DM, 1]) for m in range(Mc)]
        wa1m_s = [load(f"wa1m{m}", di["wa1m"].ap()[m], [DM, 1], F32R) for m in range(Mc)]
        ba1m_s = [load(f"ba1m{m}", di["ba1m"].ap()[m:m + 1, :], [1, 1]) for m in range(Mc)]
        wa2m_s = [load(f"wa2m{m}", di["wa2m"].ap()[m:m + 1, :], [1, 1]) for m in range(Mc)]
        ba2m_s = [load(f"ba2m{m}", di["ba2m"].ap()[m:m + 1, :], [1, 1]) for m in range(Mc)]
        br_sb = load("br", di["br"].ap(), [DRc, 1])
        gr_sb = load("gr", di["gr"].ap(), [DRc, 1])
        ber_sb = load("ber", di["ber"].ap(), [DRc, 1])
        wa1r_sb = load("wa1r", di["wa1r"].ap(), [DRc, 1], F32R)
        ba1r_sb = load("ba1r", di["ba1r"].ap(), [1, 1])
        wa2r_sb = load("wa2r", di["wa2r"].ap(), [1, 1])
        ba2r_sb = load("ba2r", di["ba2r"].ap(), [1, 1])


        # ---------------- leaf BN -> h_l, in place (f32r view) ------------
        h_sb = []
        mv_l, ve_l, sd_l, schl = [], [], [], []
        for i in range(8):
            mv, ve = bn_var_eps(stl[i], RB[i][1], f"l{i}")
            mv_l.append(mv)
            ve_l.append(ve)
        for i in range(8):
            sd_l.append(bn_sqrt(ve_l[i], RB[i][1], f"l{i}"))
        for i in range(8):
            schl.append(bn_finish(mv_l[i], sd_l[i], gl_s[i], bel_s[i],
                                  RB[i][1], f"l{i}"))
        for c in range(4):
            for i in range(8):
                sc, sh = schl[i]
                cc0 = 512 * c
                nc.vector.tensor_scalar(t_sb[i][:, cc0:cc0 + 512],
                                        t_sb[i][:, cc0:cc0 + 512], sc, sh,
                                        op0=OP.mult, op1=OP.add)
        for i in range(8):
            h_sb.append(t_sb[i])

        # ---------------- mid level ----------------
        p_m = tc.alloc_tile_pool(name="mid", bufs=1)
        p_dram = tc.alloc_tile_pool(name="dram", bufs=1, space="DRAM")
        ccin = p_dram.tile([Mc * DM, B], F32, name="ccin")
        ccout = p_dram.tile([M * DM, B], F32, addr_space="Shared", name="ccout")
        pp2 = tc.alloc_tile_pool(name="pp2", bufs=1, space="PSUM")

        hmr = []
        last_ccin = None
        for m in range(Mc):
            tm = p_m.tile([DM, B], F32R, tag="tm", bufs=2, name=f"tm{m}")
            stm = bn_alloc_stats(DM, f"m{m}")
            for b in range(NB2):
                c0 = CW2 * b
                psm = pp2.tile([DM, CW2], F32, tag="pm", bufs=2, name="psm")
                for j in range(4):
                    nc.tensor.matmul(psm, wm_sb[(m, j)],
                                     h_sb[4 * m + j][:, c0:c0 + CW2],
                                     start=(j == 0), stop=(j == 3))
                nc.scalar.activation(tm[:, c0:c0 + CW2], psm, AF.Tanh, bias=bm_s[m])
                bn_chunk_stats(stm, tm, b)
            mv, ve = bn_var_eps(stm, DM, f"m{m}")
            sd = bn_sqrt(ve, DM, f"m{m}")
            sc, sh = bn_finish(mv, sd, gm_s[m], bem_s[m], DM, f"m{m}")
            for b in range(NB2):
                c0 = CW2 * b
                nc.vector.tensor_scalar(tm[:, c0:c0 + CW2], tm[:, c0:c0 + CW2],
                                        sc, sh, op0=OP.mult, op1=OP.add)
                last_ccin = nc.sync.dma_start(
                    ccin[DM * m:DM * (m + 1), c0:c0 + CW2].bitcast(F32R),
                    tm[:, c0:c0 + CW2])
            hmr.append(tm)

        nc.gpsimd.collective_compute(
            "AllGather", OP.bypass, replica_groups=rg,
            ins=[ccin.opt()], outs=[ccout.opt()],
        )

        # root weights load early (overlaps the AllGather)
        p_wr = tc.alloc_tile_pool(name="wr", bufs=1)
        wr_sb = [p_wr.tile([128, DRc], F32R, tag=f"wrk{k}", name=f"wrk{k}")
                 for k in range(9)]
        for k in range(9):
            nc.sync.dma_start(wr_sb[k], di["wr"].ap()[k])

        # ---------------- deferred outputs + aux (overlap the AllGather) --
        for i, (r0, sz) in enumerate(RB):
            dd = nc.scalar.dma_start(di["hl"].ap()[r0:r0 + sz, :].bitcast(F32R),
                                     h_sb[i])
            add_dep_helper(dd.ins, last_ccin.ins, sync=True,
                           reason="outputs queue after collective feeders")
        for m in range(Mc):
            dd = nc.scalar.dma_start(
                di["hm"].ap()[DM * m:DM * (m + 1), :].bitcast(F32R), hmr[m])
            add_dep_helper(dd.ins, last_ccin.ins, sync=True,
                           reason="outputs queue after collective feeders")

        aux1l = p_m.tile([Lc, B], F32, tag="aux1l", name="aux1l")
        for b in range(NB2):
            c0 = CW2 * b
            psa = pp2.tile([Lc, CW2], F32, tag="pa", bufs=2, name="psa")
            for i in range(8):
                amm = nc.tensor.matmul(psa, wa1_sb[i], h_sb[i][:, c0:c0 + CW2],
                                       start=(i == 0), stop=(i == 7))
                if b == 0 and i == 0:
                    add_dep_helper(amm.ins, last_ccin.ins, sync=True,
                                   reason="aux fills the AllGather window")
            nc.scalar.activation(aux1l[:, c0:c0 + CW2], psa, AF.Tanh, bias=ba1l_sb)
        auxl32 = p_m.tile([Lc, B], F32, tag="auxl32", name="auxl32")
        nc.vector.tensor_scalar(auxl32, aux1l, wa2l_sb, ba2l_sb,
                                op0=OP.mult, op1=OP.add)
        nc.scalar.dma_start(di["auxl"].ap(), auxl32)

        for m in range(Mc):
            aux1m = p_m.tile([1, B], F32, tag="aux1m", bufs=1, name="aux1m")
            for b in range(NB2):
                c0 = CW2 * b
                psam = pp2.tile([1, CW2], F32, tag="pam", bufs=2, name="psam")
                nc.tensor.matmul(psam, wa1m_s[m], hmr[m][:, c0:c0 + CW2],
                                 start=True, stop=True)
                nc.scalar.activation(aux1m[:, c0:c0 + CW2], psam, AF.Tanh,
                                     bias=ba1m_s[m])
            auxm32 = p_m.tile([1, B], F32, tag="auxm32", bufs=1, name="auxm32")
            nc.vector.tensor_scalar(auxm32, aux1m, wa2m_s[m], ba2m_s[m],
                                    op0=OP.mult, op1=OP.add)
            nc.scalar.dma_start(di["auxm"].ap()[m:m + 1, :], auxm32)

        pp2.release()
        p_th.release()

        # ---------------- root level (feature-sharded) ----------------
        p_rt = tc.alloc_tile_pool(name="rt", bufs=1)
        pp3 = tc.alloc_tile_pool(name="pp3", bufs=1, space="PSUM")

        tr = p_rt.tile([DRc, B], F32R, tag="tr", name="tr")
        str_ = bn_alloc_stats(DRc, "r")
        psrs = [pp3.tile([DRc, CW2], F32, tag="pr", bufs=4, name=f"psr{b}")
                for b in range(NB2)]
        for ki in range(9):
            hma = p_rt.tile([128, B], F32R, tag=f"hma{ki}", bufs=1,
                            name=f"hma{ki}")
            eng = nc.sync if ki % 2 == 0 else nc.scalar
            eng.dma_start(hma, ccout[128 * ki:128 * (ki + 1), :].bitcast(F32R))
            for b in range(NB2):
                c0 = CW2 * b
                nc.tensor.matmul(psrs[b], wr_sb[ki], hma[:, c0:c0 + CW2],
                                 start=(ki == 0), stop=(ki == 8))
        for b in range(NB2):
            c0 = CW2 * b
            nc.scalar.activation(tr[:, c0:c0 + CW2], psrs[b], AF.Tanh, bias=br_sb)
            bn_chunk_stats(str_, tr, b)
        mv, ve = bn_var_eps(str_, DRc, "r")
        sd = bn_sqrt(ve, DRc, "r")
        sc, sh = bn_finish(mv, sd, gr_sb, ber_sb, DRc, "r")
        nc.vector.tensor_scalar(tr, tr, sc, sh, op0=OP.mult, op1=OP.add)
        hrr = tr

        # aux_r partial + AllReduce trigger first; h_r output DMA overlaps it
        arp = p_rt.tile([1, B], F32, tag="arp", name="arp")
        for b in range(NB2):
            c0 = CW2 * b
            psar = pp3.tile([1, CW2], F32, tag="par", bufs=2, name="psar")
            nc.tensor.matmul(psar, wa1r_sb, hrr[:, c0:c0 + CW2],
                             start=True, stop=True)
            nc.vector.tensor_copy(arp[:, c0:c0 + CW2], psar)
        arin = p_dram.tile([1, B], F32, name="arin")
        arout = p_dram.tile([1, B], F32, addr_space="Shared", name="arout")
        nc.sync.dma_start(arin, arp)
        nc.gpsimd.collective_compute(
            "AllReduce", OP.add, replica_groups=rg,
            ins=[arin.opt()], outs=[arout.opt()],
        )
        nc.scalar.dma_start(di["hr"].ap().bitcast(F32R), hrr)
        arsb = p_rt.tile([1, B], F32, tag="arsb", name="arsb")
        nc.sync.dma_start(arsb, arout)
        aux1r = p_rt.tile([1, B], F32, tag="aux1r", name="aux1r")
        nc.scalar.activation(aux1r, arsb, AF.Tanh, bias=ba1r_sb)
        auxr32 = p_rt.tile([1, B], F32, tag="auxr32", name="auxr32")
        nc.vector.tensor_scalar(auxr32, aux1r, wa2r_sb, ba2r_sb,
                                op0=OP.mult, op1=OP.add)
        nc.scalar.dma_start(di["auxr"].ap(), auxr32)

        # Lowest-priority keep-warm matmuls: the scheduler slots these into
        # PE-idle windows (BN chain, AllGather, AllReduce waits) so the HAM
        # clock gate stays open; they do no useful work.
        warm_in = p_w.tile([120, LGc], F32R, tag="warm_in", name="warm_in")
        nc.sync.dma_start(warm_in, di["wd"].ap()[0:120, :])
        for wi in range(60):
            wmm = nc.tensor.matmul(warm_ps[:, 0:LGc], wla_sb[0], warm_in,
                                   start=True, stop=True)
            if wi == 0:
                add_dep_helper(wmm.ins, last_z_mm.ins, sync=True,
                               reason="warm fillers start after phase-1")
        warm_sb = p_w.tile([128, 128], F32, tag="warm_sb", name="warm_sb")
        nc.vector.tensor_copy(warm_sb, warm_ps[:, 0:128])
        nc.scalar.dma_start(di["wrm"].ap(), warm_sb)

        pp3.release()
        pp_warm.release()
        p_rt.release()
        p_wr.release()
        p_dram.release()
        p_m.release()
        p_w.release()

    nc.compile()
    return nc


def _prep_in_maps(inputs):
    f32 = np.float32
    x = np.asarray(inputs["x"], f32)
    Wd = np.asarray(inputs["Wd"], f32)
    bd = np.asarray(inputs["bd"], f32)
    Wl = np.asarray(inputs["Wl"], f32)
    bl = np.asarray(inputs["bl"], f32)
    gl = np.asarray(inputs["gl"], f32)
    betal = np.asarray(inputs["betal"], f32)
    Wa1l = np.asarray(inputs["Wa1l"], f32)
    ba1l = np.asarray(inputs["ba1l"], f32)
    Wa2l = np.asarray(inputs["Wa2l"], f32)
    ba2l = np.asarray(inputs["ba2l"], f32)
    Wm = np.asarray(inputs["Wm"], f32)
    bm = np.asarray(inputs["bm"], f32)
    gm = np.asarray(inputs["gm"], f32)
    betam = np.asarray(inputs["betam"], f32)
    Wa1m = np.asarray(inputs["Wa1m"], f32)
    ba1m = np.asarray(inputs["ba1m"], f32)
    Wa2m = np.asarray(inputs["Wa2m"], f32)
    ba2m = np.asarray(inputs["ba2m"], f32)
    Wr = np.asarray(inputs["Wr"], f32)
    br = np.asarray(inputs["br"], f32)
    gr = np.asarray(inputs["gr"], f32)
    betar = np.asarray(inputs["betar"], f32)
    Wa1r = np.asarray(inputs["Wa1r"], f32)
    ba1r = np.asarray(inputs["ba1r"], f32)
    Wa2r = np.asarray(inputs["Wa2r"], f32)
    ba2r = np.asarray(inputs["ba2r"], f32)

    xt_full = np.zeros((NFP, B), f32)
    xt_full[:NF] = x.T
    # [NFP, B] -> [nb, 128, kt, cw] so each chunk DMA is contiguous per partition
    xt = np.ascontiguousarray(
        xt_full.reshape(KT1, 128, NB1, CW1).transpose(2, 1, 0, 3))

    # root weights, feature-padded to 904 and column-sharded
    Wr_p = np.zeros((M * DM, DRp), f32)
    Wr_p[:, :DR] = Wr
    br_p = np.zeros(DRp, f32)
    br_p[:DR] = br
    gr_p = np.ones(DRp, f32)
    gr_p[:DR] = gr
    ber_p = np.zeros(DRp, f32)
    ber_p[:DR] = betar
    wa1r_p = np.zeros((DRp, 1), f32)
    wa1r_p[:DR] = Wa1r

    in_maps = []
    for core in range(N_CORES):
        l0 = Lc * core
        m0 = Mc * core
        sl = slice(l0, l0 + Lc)
        sm = slice(m0, m0 + Mc)

        wd = np.zeros((NFP, LGc), f32)
        wd[:NF] = Wd[sl].transpose(1, 0, 2).reshape(NF, LGc)

        # leaf Linear as 4 block-diagonal [120 -> 180] matmuls, split 128+52
        wla = np.zeros((4, 120, 128), f32)
        wlb = np.zeros((4, 120, 52), f32)
        for mt in range(4):
            blk = np.zeros((120, 180), f32)
            for j in range(12):
                blk[10 * j:10 * (j + 1), 15 * j:15 * (j + 1)] = Wl[l0 + 12 * mt + j]
            wla[mt] = blk[:, :128]
            wlb[mt] = blk[:, 128:]

        # combined bias: (bd @ Wl + bl) per leaf, flattened (leaf, dl)
        blp = (np.einsum("lg,lgd->ld", bd[sl], Wl[sl]) + bl[sl]).reshape(LDc, 1)

        # aux-1 weights as a [720, 48] block matrix, row-blocked per RB
        W48 = np.zeros((LDc, Lc), f32)
        for j in range(Lc):
            W48[DL * j:DL * (j + 1), j] = Wa1l[l0 + j, :, 0]
        wa1a = np.zeros((4, 128, Lc), f32)
        wa1b = np.zeros((4, 52, Lc), f32)
        for i, (r0, sz) in enumerate(RB):
            if i % 2 == 0:
                wa1a[i // 2] = W48[r0:r0 + sz]
            else:
                wa1b[i // 2] = W48[r0:r0 + sz]

        wma = np.zeros((Mc, 2, 128, DM), f32)
        wmb = np.zeros((Mc, 2, 52, DM), f32)
        for m in range(Mc):
            wmat = Wm[m0 + m]  # [360, 72]
            wma[m, 0] = wmat[0:128]
            wmb[m, 0] = wmat[128:180]
            wma[m, 1] = wmat[180:308]
            wmb[m, 1] = wmat[308:360]

        rs = slice(DRc * core, DRc * (core + 1))
        in_maps.append({
            "xt": xt,
            "wd": wd,
            "wla": wla,
            "wlb": wlb,
            "blp": blp,
            "gl": gl[sl].reshape(LDc, 1).astype(f32),
            "bel": betal[sl].reshape(LDc, 1).astype(f32),
            "wa1a": wa1a,
            "wa1b": wa1b,
            "ba1l": ba1l[sl].reshape(Lc, 1),
            "wa2l": Wa2l[sl, 0, :].reshape(Lc, 1),
            "ba2l": ba2l[sl].reshape(Lc, 1),
            "wma": wma,
            "wmb": wmb,
            "bm": bm[sm].reshape(Mc * DM, 1),
            "gm": gm[sm].reshape(Mc * DM, 1),
            "bem": betam[sm].reshape(Mc * DM, 1),
            "wa1m": Wa1m[sm],
            "ba1m": ba1m[sm].reshape(Mc, 1),
            "wa2m": Wa2m[sm, 0, :].reshape(Mc, 1),
            "ba2m": ba2m[sm].reshape(Mc, 1),
            "wr": np.ascontiguousarray(Wr_p[:, rs]).reshape(9, 128, DRc),
            "br": br_p[rs].reshape(DRc, 1),
            "gr": gr_p[rs].reshape(DRc, 1),
            "ber": ber_p[rs].reshape(DRc, 1),
            "wa1r": np.ascontiguousarray(wa1r_p[rs]),
            "ba1r": ba1r.reshape(1, 1),
            "wa2r": Wa2r.reshape(1, 1)[:1, :1],
            "ba2r": ba2r.reshape(1, 1),
        })
    return in_maps


def _get_program():
    if "nc" not in _CACHE:
        _CACHE["nc"] = _build_program()
    return _CACHE["nc"]


def kernel(**inputs):
    nc = _get_program()
    in_maps = _prep_in_maps(inputs)
    res = bass_utils.run_bass_kernel_spmd(nc, in_maps,
                                          core_ids=list(range(N_CORES)))
    rs = res.results
    f32 = np.float32

    h_l = np.concatenate([r["hl"].reshape(Lc, DL, B) for r in rs], 0)
    h_l = np.ascontiguousarray(h_l.transpose(0, 2, 1), f32)
    aux_l = np.ascontiguousarray(
        np.concatenate([r["auxl"] for r in rs], 0), f32)[:, :, None]
    h_m = np.concatenate([r["hm"].reshape(Mc, DM, B) for r in rs], 0)
    h_m = np.ascontiguousarray(h_m.transpose(0, 2, 1), f32)
    aux_m = np.ascontiguousarray(
        np.concatenate([r["auxm"] for r in rs], 0), f32)[:, :, None]
    h_r = np.concatenate([r["hr"] for r in rs], 0)[:DR]
    h_r = np.ascontiguousarray(h_r.T, f32)
    aux_r = np.ascontiguousarray(rs[0]["auxr"].T, f32)
    return (aux_l, aux_m, aux_r, h_l, h_m, h_r)
